# revision 11
# baseline (speedup 1.0000x reference)
"""Trainium2 Bass kernel for nn_CrossAttention (cross-attention + GEGLU MLP).

Sharding over 8 NeuronCores: core c -> batch b = c//4, lane l = c%4.
Within a 4-core group (one batch): tensor-parallel over heads for
QKV/attention/out-proj (4 heads per core); two token-chunked
ReduceScatters hand each lane a 512-token set (one 128-token quarter of
each 512-token attention q-block); the MLP runs data-parallel on that
slice with the full 8192 hidden dim, in two 256-token half-passes.

The kernel is paced by the Scalar-engine exp stream (softmax numerator,
~256 x [128,1024]-col ACTIVATEs): attention q-blocks are 512 wide so
PSUM fits score tiles for both head-pairs plus the attention outputs
plus a rotating [128,512] aux pool that serves QKV projection, out-proj,
W1/W2 and LayerNorm stats without ever blocking the score->exp->attnV
pipeline. MLP pass A (W1 matmuls, gelu deferred) is interleaved between
attention t-steps of the last three units; gelu-A + W2-A run under
ReduceScatter B; only phase_d(B) + W1-B + W2-B trail the exp stream.

Host-side folding: LayerNorm-1 stats are computed on host and folded
into transposed fp8 activations plus bf16 augmented contraction rows;
1/sqrt(dh) and all fp8 range scales fold into weights; descales ride
the exp/gelu `scale` parameter and PSUM-eviction multiplies. Softmax
skips max-subtraction; denominators ride attention@V as a 65th ones
column of V; 1/Z uses the fast custom-DVE reciprocal.
"""
import numpy as np
import ml_dtypes

import concourse.bass as bass
import concourse.mybir as mybir
import concourse.tile as tile
from concourse import bacc
from concourse.bass_utils import run_bass_kernel_spmd

f32 = mybir.dt.float32
bf16 = mybir.dt.bfloat16
f8 = mybir.dt.float8e4
AF = mybir.ActivationFunctionType
ALU = mybir.AluOpType
DR = mybir.MatmulPerfMode.DoubleRow

N_CORES = 8
GROUPS = [[0, 1, 2, 3], [4, 5, 6, 7]]
B, NQ, NKV, D = 2, 2048, 4096, 1024
H, DH = 16, 64
HID = 8192
EPS = 1e-6
HL = 4            # heads per core
EL = HL * DH      # local head channels = 256
TL = NQ // 4      # MLP token slice per lane = 512
QT = TL // 2      # MLP half-pass tokens = 256
QB = 512          # attention q block (4 blocks)
QQ = QB // 4      # per-lane tokens per q block = 128
P = 128

NKV_T = NKV // P  # 32 kv token tiles
DK = D // P       # 8 contraction tiles
VS = 68           # per-head stride in v_sb (64 + ones col + pad to %16)

# fp8 range scales (folded into weights on host, undone at eviction/exp)
SQ, SK, SV, SOT, SWO = 64.0, 16.0, 8.0, 64.0, 8.0
# MLP scales: h0 fp8 = h0*SH0; w1a fp8 = W1a*(SH2/SH0); h2t fp8 = h2*SH2
S1, S2 = 8.0, 32.0        # bf16-MLP path scales (as baseline)
SH0, SH2, SG = 16.0, 32.0, 8.0  # e4m3 max finite is 240: keep h2*SH2 under it

MLP_F8 = True             # g-path W1g + h0 in fp8 (DoubleRow)
MLP_DT = f8 if MLP_F8 else bf16
W1A_F8 = False            # a-path bf16: a errors hit h2 unsquashed
W1A_DT = f8 if (MLP_F8 and W1A_F8) else bf16
W2_F8 = False             # h2t/W2 stay bf16: halves MLP error at +13us tail
W2_DT = f8 if W2_F8 else bf16
H2_DT = f8 if (MLP_F8 and W2_F8) else bf16
DEBUG = False
RT2PI = 0.7978845608028654  # sqrt(2/pi) for tanh-form gelu


def build_kernel(n_iters=1):
    nc = bacc.Bacc("TRN2", target_bir_lowering=False, debug=False,
                   num_devices=N_CORES)
    # ---- per-core external I/O
    xqT = nc.dram_tensor("xqT", [D, NQ], f8, kind="ExternalInput")
    xkvT = nc.dram_tensor("xkvT", [D, NKV], f8, kind="ExternalInput")
    augr_q = nc.dram_tensor("augr_q", [2, NQ], bf16, kind="ExternalInput")
    augr_kv = nc.dram_tensor("augr_kv", [2, NKV], bf16, kind="ExternalInput")
    wq = nc.dram_tensor("wq", [P, 4, 2, EL], f8, kind="ExternalInput")
    wk = nc.dram_tensor("wk", [P, 4, 2, EL], f8, kind="ExternalInput")
    wv = nc.dram_tensor("wv", [P, 4, 2, EL], f8, kind="ExternalInput")
    aug_q = nc.dram_tensor("aug_q", [2, EL], bf16, kind="ExternalInput")
    aug_k = nc.dram_tensor("aug_k", [2, EL], bf16, kind="ExternalInput")
    aug_v = nc.dram_tensor("aug_v", [2, EL], bf16, kind="ExternalInput")
    wo = nc.dram_tensor("wo", [P, 2, D], f8, kind="ExternalInput")
    bo_pc = nc.dram_tensor("bo_pc", [P, DK], f32, kind="ExternalInput")
    xres_T = nc.dram_tensor("xres_T", [D, TL], bf16, kind="ExternalInput")
    w1a_t = nc.dram_tensor("w1a_t", [32, P, 4, 2, P], W1A_DT, kind="ExternalInput")
    w1g_t = nc.dram_tensor("w1g_t", [32, P, 4, 2, P], MLP_DT, kind="ExternalInput")
    b1a_pc = nc.dram_tensor("b1a_pc", [P, 32], f32, kind="ExternalInput")
    b1g_pc = nc.dram_tensor("b1g_pc", [P, 32], f32, kind="ExternalInput")
    w2_t = nc.dram_tensor("w2_t", [DK, P, 16, 2, P], W2_DT, kind="ExternalInput")
    b2row_d = nc.dram_tensor("b2row", [1, D], bf16, kind="ExternalInput")
    out = nc.dram_tensor("out", [D, TL], f32, kind="ExternalOutput")
    dbg = {}
    if DEBUG:
        dbg["qT"] = nc.dram_tensor("dbg_qT", [P, 2, NQ], bf16, kind="ExternalOutput")
        dbg["kT"] = nc.dram_tensor("dbg_kT", [P, 2, NKV], bf16, kind="ExternalOutput")
        dbg["v"] = nc.dram_tensor("dbg_v", [P, NKV_T, HL * VS], f8, kind="ExternalOutput")
        dbg["oT"] = nc.dram_tensor("dbg_oT", [P, 2, NQ], f8, kind="ExternalOutput")
        dbg["rec"] = nc.dram_tensor("dbg_rec", [1, 2, QB], f32, kind="ExternalOutput")
        dbg["h0"] = nc.dram_tensor("dbg_h0", [P, DK, TL], MLP_DT, kind="ExternalOutput")
        dbg["h2t"] = nc.dram_tensor("dbg_h2t", [P, 32, TL], H2_DT, kind="ExternalOutput")
        dbg["xr"] = nc.dram_tensor("dbg_xr", [P, DK, TL], bf16, kind="ExternalOutput")

    inv12 = 1.0 / (SH2 * S2) if MLP_F8 else 1.0 / (S1 * S2)

    with tile.TileContext(nc) as tc:
        def body(_iv=None):
            from contextlib import ExitStack
            with (
                tc.tile_pool(name="persist", bufs=1) as pp,
                tc.tile_pool(name="dram", bufs=1, space="DRAM") as dram,
                tc.tile_pool(name="sc", bufs=1, space="PSUM") as sc,
                tc.tile_pool(name="po", bufs=1, space="PSUM") as po,
                tc.tile_pool(name="aux", bufs=2, space="PSUM") as aux,
                tc.tile_pool(name="asb", bufs=3) as asb,
                tc.tile_pool(name="xin", bufs=2) as xin,
                tc.tile_pool(name="wdma", bufs=2) as wdma,
                tc.tile_pool(name="mx", bufs=1) as mx,
                tc.tile_pool(name="md", bufs=2) as md,
            ):
                ones_row = pp.tile([1, P], bf16)
                nc.any.memset(ones_row[:], 1.0)
                ones_rq = pp.tile([1, QT], bf16)
                nc.any.memset(ones_rq[:], 1.0)
                ones_col = pp.tile([P, 1], bf16)
                nc.any.memset(ones_col[:], 1.0)
                e8row = pp.tile([1, P], bf16)  # lhsT for 1/Z broadcasts
                nc.any.memset(e8row[:], SOT / SV)
                inv12_c = pp.tile([P, 1], f32)
                nc.any.memset(inv12_c[:], inv12)

                # small/static inputs
                wq_sb = pp.tile([P, 4, 2, EL], f8)
                nc.sync.dma_start(wq_sb[:], wq[:])
                wk_sb = pp.tile([P, 4, 2, EL], f8)
                nc.sync.dma_start(wk_sb[:], wk[:])
                wv_sb = pp.tile([P, 4, 2, EL], f8)
                nc.sync.dma_start(wv_sb[:], wv[:])
                augq_sb = pp.tile([2, EL], bf16)
                nc.sync.dma_start(augq_sb[:], aug_q[:])
                augk_sb = pp.tile([2, EL], bf16)
                nc.sync.dma_start(augk_sb[:], aug_k[:])
                augv_sb = pp.tile([2, EL], bf16)
                nc.sync.dma_start(augv_sb[:], aug_v[:])
                wo_sb = pp.tile([P, 2, D], f8)
                nc.sync.dma_start(wo_sb[:], wo[:])
                augrq_sb = pp.tile([2, NQ], bf16)
                nc.sync.dma_start(augrq_sb[:], augr_q[:])
                augrkv_sb = pp.tile([2, NKV], bf16)
                nc.sync.dma_start(augrkv_sb[:], augr_kv[:])

                qT = pp.tile([P, 2, NQ], bf16)      # [parity*64+dh, hpair, t]
                kT = pp.tile([P, 2, NKV], bf16)
                v_sb = pp.tile([P, NKV_T, HL * VS], f8)
                nc.any.memset(v_sb[:], 1.0)         # ones cols for denominators
                oT = pp.tile([P, 2, NQ], f8)

                bo_sb = pp.tile([P, DK], f32)
                nc.sync.dma_start(bo_sb[:], bo_pc[:])
                b1a_sb = pp.tile([P, 32], f32)
                nc.sync.dma_start(b1a_sb[:], b1a_pc[:])
                b1g_sb = pp.tile([P, 32], f32)
                nc.sync.dma_start(b1g_sb[:], b1g_pc[:])
                b2row = pp.tile([1, D], bf16)
                nc.sync.dma_start(b2row[:], b2row_d[:])

                rs_inA = dram.tile([4, D, QT], bf16)
                rs_outA = dram.tile([D, QT], bf16)
                rs_inB = dram.tile([4, D, QT], bf16)
                rs_outB = dram.tile([D, QT], bf16)

                # MLP state (coexists with attention for pass-A overlap)
                # x residual in bf16 (abs err ~0.6% of |x| << gate)
                x_r = mx.tile([P, DK, TL], bf16, name="x_r")
                h0 = mx.tile([P, DK, TL], MLP_DT, name="h0")
                h0b = (mx.tile([P, DK, TL], bf16, name="h0b")
                       if (MLP_F8 and not W1A_F8) else None)
                h2t = mx.tile([P, 32, TL], H2_DT, name="h2t")

                eps_row = pp.tile([1, 1], f32)
                nc.any.memset(eps_row[:], EPS)

                # ---------------- building blocks ----------------
                def qproj_block(qb):
                    """Q projection of 512 tokens of block qb -> qT."""
                    xq_ck = xin.tile([P, DK, QB], f8, tag="xq")
                    nc.sync.dma_start(
                        xq_ck[:],
                        xqT[:, qb * QB:(qb + 1) * QB].rearrange(
                            "(kt p) t -> p kt t", p=P))
                    for mt in range(2):
                        ps = aux.tile([P, QB], f32, tag="a")
                        for ktp in range(4):
                            nc.tensor.matmul(
                                ps[:], wq_sb[:, ktp, :, mt * P:(mt + 1) * P],
                                xq_ck[:, 2 * ktp:2 * ktp + 2, :],
                                start=(ktp == 0), stop=False, perf_mode=DR)
                        nc.tensor.matmul(
                            ps[:], augq_sb[:, mt * P:(mt + 1) * P],
                            augrq_sb[:, qb * QB:(qb + 1) * QB],
                            start=False, stop=True)
                        nc.vector.tensor_copy(
                            qT[:, mt, qb * QB:(qb + 1) * QB], ps[:])

                def kproj_chunk(ck, xkv_ck):
                    ts0 = ck * 512
                    for mt in range(2):
                        ps = aux.tile([P, QB], f32, tag="a")
                        for ktp in range(4):
                            nc.tensor.matmul(
                                ps[:], wk_sb[:, ktp, :, mt * P:(mt + 1) * P],
                                xkv_ck[:, 2 * ktp:2 * ktp + 2, :],
                                start=(ktp == 0), stop=False, perf_mode=DR)
                        nc.tensor.matmul(
                            ps[:], augk_sb[:, mt * P:(mt + 1) * P],
                            augrkv_sb[:, ts0:ts0 + 512],
                            start=False, stop=True)
                        nc.vector.tensor_copy(kT[:, mt, ts0:ts0 + 512], ps[:])

                def vproj_chunk(ck, xkv_ck):
                    """Token-major V projection of 512 kv tokens."""
                    for r in range(2):
                        ps = aux.tile([P, QB], f32, tag="a")
                        for t2 in range(2):
                            tt = 2 * r + t2
                            cs = slice(t2 * EL, (t2 + 1) * EL)
                            lms = slice(tt * P, (tt + 1) * P)
                            for ktp in range(4):
                                nc.tensor.matmul(
                                    ps[:, cs],
                                    xkv_ck[:, 2 * ktp:2 * ktp + 2, lms],
                                    wv_sb[:, ktp, :, :],
                                    start=(ktp == 0), stop=False, perf_mode=DR)
                            nc.tensor.matmul(
                                ps[:, cs],
                                augrkv_sb[:, ck * 512 + tt * P:
                                          ck * 512 + (tt + 1) * P],
                                augv_sb[:], start=False, stop=True)
                        kvt = 4 * ck + 2 * r
                        # strided evict: [2 tt, 4 heads, 64ch] -> v_sb
                        nc.vector.tensor_copy(
                            v_sb[:, kvt:kvt + 2, :].rearrange(
                                "p t (h c) -> p t h c", h=HL)[:, :, :, 0:DH],
                            ps[:].rearrange("p (t h c) -> p t h c", t=2, h=HL))

                def scores_t(qb, hp, t):
                    sps = []
                    for h2 in range(2):
                        pp_ = slice(h2 * DH, (h2 + 1) * DH)
                        s_ps = sc.tile([P, 2, QB], f32, tag=f"s{h2}")
                        sps.append(s_ps)
                        for i in range(2):
                            kvt = 2 * t + i
                            nc.tensor.matmul(
                                s_ps[:, i, :],
                                kT[pp_, hp, kvt * P:(kvt + 1) * P],
                                qT[pp_, hp, qb * QB:(qb + 1) * QB],
                                start=True, stop=True)
                    return sps

                def exp_t(sps):
                    p2s = []
                    for h2 in range(2):
                        p2 = asb.tile([P, 2, QB], f8, tag=f"p{h2}")
                        p2s.append(p2)
                        nc.scalar.activation(p2[:], sps[h2][:],
                                             AF.Exp, scale=1.0 / (SQ * SK))
                    return p2s

                def attnv_t(hp, t, o_ps, p2s):
                    for h2 in range(2):
                        h = 2 * hp + h2
                        nc.tensor.matmul(
                            o_ps[h2][:],
                            v_sb[:, 2 * t:2 * t + 2, h * VS:h * VS + DH + 1],
                            p2s[h2][:],
                            start=(t == 0), stop=(t == 15), perf_mode=DR)

                def attn_finish(qb, hp, o_ps):
                    qs = slice(qb * QB, (qb + 1) * QB)
                    # custom-DVE ops misread PSUM at partition offset 64:
                    # stage the z rows to SBUF before the fast reciprocal
                    zrow = asb.tile([1, 2, QB], f32, tag="zrow", bufs=1)
                    for h2 in range(2):
                        nc.vector.tensor_copy(zrow[0:1, h2, :],
                                              o_ps[h2][DH:DH + 1, :])
                    rec = asb.tile([1, 2, QB], f32, tag="rec", bufs=1)
                    nc.vector.reciprocal_approx_fast(rec[:], zrow[:])
                    if DEBUG:
                        nc.sync.dma_start(dbg["rec"][:], rec[:])
                    rec_bf = asb.tile([1, 2, QB], bf16, tag="recbf", bufs=1)
                    nc.vector.tensor_copy(rec_bf[:], rec[:])
                    rc2 = aux.tile([P, QB], f32, tag="a")
                    for h2 in range(2):
                        nc.tensor.matmul(
                            rc2[h2 * DH:(h2 + 1) * DH, :],
                            e8row[0:1, 0:DH], rec_bf[0:1, h2, :],
                            start=True, stop=True)
                    rc2_sb = asb.tile([P, QB], bf16, tag="rc", bufs=2)
                    nc.vector.tensor_copy(rc2_sb[:], rc2[:])
                    for h2 in range(2):
                        hs = slice(h2 * DH, (h2 + 1) * DH)
                        nc.vector.tensor_tensor(
                            oT[hs, hp, qs], o_ps[h2][0:DH, :], rc2_sb[hs, :],
                            ALU.mult)

                def proj_rs_block(qb, rs_dram):
                    cs = slice((qb % 2) * QQ, (qb % 2 + 1) * QQ)
                    for mt in range(DK):
                        ps = aux.tile([P, QB], f32, tag="a")
                        nc.tensor.matmul(
                            ps[:], wo_sb[:, :, mt * P:(mt + 1) * P],
                            oT[:, :, qb * QB:(qb + 1) * QB],
                            start=True, stop=True, perf_mode=DR)
                        stage = asb.tile([P, QB], bf16, tag="stage", bufs=2)
                        nc.vector.tensor_scalar_mul(
                            stage[:], ps[:], 1.0 / (SOT * SWO))
                        for l in range(4):
                            nc.sync.dma_start(
                                rs_dram[l, mt * P:(mt + 1) * P, cs],
                                stage[:, l * QQ:(l + 1) * QQ])

                def phase_d(half, rs_out_half):
                    ts = slice(half * QT, (half + 1) * QT)
                    rsx = md.tile([P, DK, QT], bf16, tag="rsx", bufs=1,
                                  name="rsx")
                    nc.sync.dma_start(
                        rsx[:],
                        rs_out_half[:].rearrange("(kt p) t -> p kt t", p=P))
                    xres_h = md.tile([P, DK, QT], bf16, tag="xres", bufs=1,
                                     name="xres_h")
                    nc.sync.dma_start(
                        xres_h[:],
                        xres_T[:, ts].rearrange("(kt p) t -> p kt t", p=P))
                    x2 = md.tile([P, DK, QT], bf16, tag="x2", bufs=1, name="x2")
                    for kt in range(DK):
                        nc.vector.scalar_tensor_tensor(
                            x_r[:, kt, ts], rsx[:, kt, :], bo_sb[:, kt:kt + 1],
                            xres_h[:, kt, :], ALU.add, ALU.add)
                        nc.vector.tensor_tensor(x2[:, kt, :], x_r[:, kt, ts],
                                                x_r[:, kt, ts], ALU.mult)
                    st = aux.tile([P, QB], f32, tag="a")
                    for kt in range(DK):
                        nc.tensor.matmul(st[0:1, 0:QT], ones_col[:],
                                         x_r[:, kt, ts],
                                         start=(kt == 0), stop=(kt == DK - 1))
                    for kt in range(DK):
                        nc.tensor.matmul(st[0:1, QT:2 * QT], ones_col[:],
                                         x2[:, kt, :],
                                         start=(kt == 0), stop=(kt == DK - 1))
                    mu_f = md.tile([1, QT], f32, tag="muf", bufs=1, name="mu_f")
                    nc.vector.tensor_scalar_mul(mu_f[:], st[0:1, 0:QT], 1.0 / D)
                    ex2 = md.tile([1, QT], f32, tag="ex2", bufs=1, name="ex2")
                    nc.vector.tensor_scalar_mul(ex2[:], st[0:1, QT:2 * QT],
                                                1.0 / D)
                    mu2 = md.tile([1, QT], f32, tag="mu2", bufs=1, name="mu2")
                    nc.vector.tensor_tensor(mu2[:], mu_f[:], mu_f[:], ALU.mult)
                    var = md.tile([1, QT], f32, tag="var", bufs=1, name="var")
                    nc.vector.tensor_tensor(var[:], ex2[:], mu2[:],
                                            ALU.subtract)
                    rr = md.tile([1, QT], f32, tag="rr", bufs=1, name="rr")
                    nc.scalar.activation(rr[:], var[:], AF.Sqrt,
                                         bias=eps_row[:])
                    rstd2 = md.tile([1, QT], f32, tag="rstd2", bufs=1, name="rstd2")
                    nc.vector.reciprocal_approx_fast(rstd2[:], rr[:])
                    rbf = md.tile([1, 2 * QT], bf16, tag="rbf", bufs=1, name="rbf")
                    nc.vector.tensor_copy(rbf[0:1, 0:QT], rstd2[:])
                    nc.vector.tensor_copy(rbf[0:1, QT:2 * QT], mu_f[:])
                    st2 = aux.tile([P, QB], f32, tag="a")
                    nc.tensor.matmul(st2[:, 0:QT], ones_row[:],
                                     rbf[0:1, QT:2 * QT], start=True, stop=True)
                    nc.tensor.matmul(st2[:, QT:2 * QT], ones_row[:],
                                     rbf[0:1, 0:QT], start=True, stop=True)
                    mu_bc = md.tile([P, QT], bf16, tag="mubc", bufs=1, name="mu_bc")
                    nc.vector.tensor_copy(mu_bc[:], st2[:, 0:QT])
                    r2_bc = md.tile([P, QT], bf16, tag="r2bc", bufs=1, name="r2_bc")
                    nc.vector.tensor_copy(r2_bc[:], st2[:, QT:2 * QT])
                    sh0 = SH0 if MLP_F8 else 1.0
                    for kt in range(DK):
                        t = md.tile([P, QT], bf16, tag="dt", name="dt")
                        nc.vector.tensor_tensor(t[:], x_r[:, kt, ts],
                                                mu_bc[:], ALU.subtract)
                        if MLP_F8:
                            tf = md.tile([P, QT], bf16, tag="dt2", name="dt2")
                            nc.vector.tensor_tensor(tf[:], t[:], r2_bc[:],
                                                    ALU.mult)
                            nc.vector.tensor_scalar_mul(
                                h0[:, kt, ts], tf[:], sh0)
                            if h0b is not None:
                                nc.vector.tensor_scalar_mul(
                                    h0b[:, kt, ts], tf[:], sh0)
                        else:
                            nc.vector.tensor_tensor(h0[:, kt, ts], t[:],
                                                    r2_bc[:], ALU.mult)

                GSC = (1.0 / (SH0 * SG)) if MLP_F8 else (1.0 / S1)

                def w1_j(j, half):
                    """W1 block j on one 256-token half; gelu in tanh form
                    (Tanh shares the ACT table with Exp - no table loads).
                    The 0.5 of gelu is folded into w1a/b1a on host."""
                    ts = slice(half * QT, (half + 1) * QT)
                    wa = wdma.tile([P, 4, 2, P], W1A_DT, tag="wa", name="wa")
                    nc.sync.dma_start(wa[:], w1a_t[j])
                    wg = wdma.tile([P, 4, 2, P], MLP_DT, tag="wg", name="wg")
                    nc.sync.dma_start(wg[:], w1g_t[j])
                    ag = aux.tile([P, QB], f32, tag="a")
                    h0a = h0b if h0b is not None else h0
                    if MLP_F8 and W1A_F8:
                        for ktp in range(4):
                            nc.tensor.matmul(ag[:, 0:QT], wa[:, ktp, :, :],
                                             h0[:, 2 * ktp:2 * ktp + 2, ts],
                                             start=(ktp == 0),
                                             stop=(ktp == 3), perf_mode=DR)
                    else:
                        for kt in range(DK):
                            nc.tensor.matmul(ag[:, 0:QT],
                                             wa[:, kt // 2, kt % 2, :],
                                             h0a[:, kt, ts],
                                             start=(kt == 0),
                                             stop=(kt == DK - 1))
                    if MLP_F8:
                        for ktp in range(4):
                            nc.tensor.matmul(ag[:, QT:2 * QT],
                                             wg[:, ktp, :, :],
                                             h0[:, 2 * ktp:2 * ktp + 2, ts],
                                             start=(ktp == 0),
                                             stop=(ktp == 3), perf_mode=DR)
                    else:
                        for kt in range(DK):
                            nc.tensor.matmul(ag[:, QT:2 * QT],
                                             wg[:, kt // 2, kt % 2, :],
                                             h0[:, kt, ts],
                                             start=(kt == 0),
                                             stop=(kt == DK - 1))
                    g = md.tile([P, QT], bf16, tag="g", name="g")
                    nc.vector.tensor_scalar(g[:], ag[:, QT:2 * QT], GSC,
                                            b1g_sb[:, j:j + 1],
                                            ALU.mult, ALU.add)
                    g2 = md.tile([P, QT], bf16, tag="g2", name="g2")
                    nc.vector.tensor_tensor(g2[:], g[:], g[:], ALU.mult)
                    v1 = md.tile([P, QT], bf16, tag="v1", name="v1")
                    nc.vector.tensor_scalar(v1[:], g2[:], 0.044715, 1.0,
                                            ALU.mult, ALU.add)
                    u = md.tile([P, QT], bf16, tag="u", name="u")
                    nc.vector.tensor_tensor(u[:], g[:], v1[:], ALU.mult)
                    th = md.tile([P, QT], bf16, tag="th", name="th")
                    nc.scalar.activation(th[:], u[:], AF.Tanh, scale=RT2PI)
                    w = md.tile([P, QT], bf16, tag="w", name="w")
                    nc.vector.scalar_tensor_tensor(w[:], th[:], 1.0, g[:],
                                                   ALU.add, ALU.mult)
                    nc.vector.scalar_tensor_tensor(
                        h2t[:, j, ts], ag[:, 0:QT], b1a_sb[:, j:j + 1],
                        w[:], ALU.add, ALU.mult)

                def w2_half(half):
                    ts = slice(half * QT, (half + 1) * QT)
                    for mo in range(DK):
                        y = aux.tile([P, QB], f32, tag="a")
                        for wh in range(2):
                            w2s = wdma.tile([P, 8, 2, P], W2_DT, tag="w2s",
                                            name="w2s")
                            nc.sync.dma_start(w2s[:],
                                              w2_t[mo, :, wh * 8:(wh + 1) * 8])
                            if W2_F8:
                                for k8 in range(8):
                                    ktp = wh * 8 + k8
                                    nc.tensor.matmul(
                                        y[:, 0:QT], w2s[:, k8, :, :],
                                        h2t[:, 2 * ktp:2 * ktp + 2, ts],
                                        start=(ktp == 0), stop=False,
                                        perf_mode=DR)
                            else:
                                for k16 in range(16):
                                    kt = wh * 16 + k16
                                    nc.tensor.matmul(
                                        y[:, 0:QT],
                                        w2s[:, k16 // 2, k16 % 2, :],
                                        h2t[:, kt, ts],
                                        start=(kt == 0), stop=False)
                        nc.tensor.matmul(y[:, 0:QT],
                                         b2row[0:1, mo * P:(mo + 1) * P],
                                         ones_rq[:], start=False, stop=True)
                        fin = md.tile([P, QT], f32, tag="fin", name="fin")
                        nc.vector.scalar_tensor_tensor(
                            fin[:], y[:, 0:QT], inv12_c[:], x_r[:, mo, ts],
                            ALU.mult, ALU.add)
                        nc.sync.dma_start(out[mo * P:(mo + 1) * P, ts], fin[:])

                # ---------------- schedule ----------------
                # per-(qb,hp)-unit extra work inserted between t-steps:
                #   key (qb, hp) -> dict t -> list of thunks
                def unit(qb, hp, interleave_kv=False, extra=None):
                    o_ps = [po.tile([DH + 1, QB], f32, tag=f"o{h2}",
                                    name=f"o{h2}") for h2 in range(2)]
                    for t in range(16):
                        if interleave_kv and t % 2 == 0:
                            ck = t // 2
                            xkv_ck = xin.tile([P, DK, 512], f8, tag="xkv")
                            nc.sync.dma_start(
                                xkv_ck[:],
                                xkvT[:, ck * 512:(ck + 1) * 512].rearrange(
                                    "(kt p) t -> p kt t", p=P))
                            kproj_chunk(ck, xkv_ck)
                            vproj_chunk(ck, xkv_ck)
                        sps = scores_t(qb, hp, t)
                        p2s = exp_t(sps)
                        attnv_t(hp, t, o_ps, p2s)
                        if extra is not None and t in extra:
                            for thunk in extra[t]:
                                thunk()
                    attn_finish(qb, hp, o_ps)

                qproj_block(0)
                unit(0, 0, interleave_kv=True)
                unit(0, 1, extra={4: [lambda: qproj_block(1)]})
                proj_rs_block(0, rs_inA)
                unit(1, 0)
                unit(1, 1, extra={4: [lambda: qproj_block(2)]})
                proj_rs_block(1, rs_inA)
                nc.gpsimd.collective_compute(
                    "ReduceScatter", ALU.add, replica_groups=GROUPS,
                    ins=[rs_inA[:].opt()], outs=[rs_outA[:].opt()])
                unit(2, 0)
                unit(2, 1, extra={4: [lambda: qproj_block(3)]})
                proj_rs_block(2, rs_inB)
                # phase_d(0) + W1 pass A interleaved under remaining exps
                # (RS_A lands ~mid-unit(2,1), so unit(3,0) never stalls)
                ex30 = {1: [lambda: phase_d(0, rs_outA)]}
                for t, j in zip(range(3, 16), range(0, 13)):
                    ex30.setdefault(t, []).append(lambda j=j: w1_j(j, 0))
                unit(3, 0, extra=ex30)
                ex31 = {}
                for t, j in zip(range(1, 16), range(13, 28)):
                    ex31.setdefault(t, []).append(lambda j=j: w1_j(j, 0))
                unit(3, 1, extra=ex31)
                proj_rs_block(3, rs_inB)
                nc.gpsimd.collective_compute(
                    "ReduceScatter", ALU.add, replica_groups=GROUPS,
                    ins=[rs_inB[:].opt()], outs=[rs_outB[:].opt()])
                # tail: W1-A leftovers + W2-A during RS_B, then half B
                for j in range(28, 32):
                    w1_j(j, 0)
                w2_half(0)
                phase_d(1, rs_outB)
                for j in range(32):
                    w1_j(j, 1)
                w2_half(1)
                if DEBUG:
                    nc.sync.dma_start(dbg["qT"][:], qT[:])
                    nc.sync.dma_start(dbg["kT"][:], kT[:])
                    nc.sync.dma_start(dbg["v"][:], v_sb[:])
                    nc.sync.dma_start(dbg["oT"][:], oT[:])
                    nc.sync.dma_start(dbg["h0"][:], h0[:])
                    nc.sync.dma_start(dbg["h2t"][:], h2t[:])
                    nc.sync.dma_start(dbg["xr"][:], x_r[:])

        for _ in range(n_iters):
            body()
    nc.compile()
    return nc


# ---------------------------------------------------------------------------
# Host-side sharding / folding
# ---------------------------------------------------------------------------

def lane_idx(l):
    """Token indices owned by lane l, MLP order: half A (qb0,1), half B."""
    return np.concatenate([
        np.arange(qb * QB + l * QQ, qb * QB + (l + 1) * QQ)
        for qb in range(4)])


def prepare_inputs(inputs):
    bf = lambda a: np.ascontiguousarray(a).astype(ml_dtypes.bfloat16)
    q8 = lambda a: np.ascontiguousarray(a).astype(ml_dtypes.float8_e4m3)
    f = lambda a: np.ascontiguousarray(a, dtype=np.float32)
    inp = {k: np.asarray(v, dtype=np.float32) for k, v in inputs.items()}

    Wq = inp["Wq"].reshape(D, H * DH)
    Wk = inp["Wk"].reshape(D, H * DH)
    Wv = inp["Wv"].reshape(D, H * DH)
    Wo = inp["Wo"].reshape(H * DH, D)
    bq = inp["bq"].reshape(H * DH)
    bk = inp["bk"].reshape(H * DH)
    bv = inp["bv"].reshape(H * DH)
    rdh = 1.0 / np.sqrt(DH)

    Wq2 = inp["ln_q_scale"][:, None] * Wq * (SQ * rdh)
    bq2 = (bq + inp["ln_q_bias"] @ Wq) * (SQ * rdh)
    Wk2 = inp["ln_kv_scale"][:, None] * Wk * SK
    bk2 = (bk + inp["ln_kv_bias"] @ Wk) * SK
    Wv2 = inp["ln_kv_scale"][:, None] * Wv * SV
    bv2 = (bv + inp["ln_kv_bias"] @ Wv) * SV

    def stats(x):
        mu = x.mean(-1)
        var = x.var(-1)
        rstd = 1.0 / np.sqrt(var + EPS)
        return mu, rstd

    mu_q, rstd_q = stats(inp["inputs_q"])     # [B, NQ]
    mu_kv, rstd_kv = stats(inp["inputs_kv"])  # [B, NKV]
    xqT_all = [q8((inp["inputs_q"][b] * rstd_q[b][:, None]).T) for b in range(B)]
    xkvT_all = [q8((inp["inputs_kv"][b] * rstd_kv[b][:, None]).T) for b in range(B)]
    augr_q_all = [bf(np.stack([mu_q[b] * rstd_q[b], np.ones(NQ, np.float32)]))
                  for b in range(B)]
    augr_kv_all = [bf(np.stack([mu_kv[b] * rstd_kv[b], np.ones(NKV, np.float32)]))
                   for b in range(B)]

    def dr4(w):   # [D, M] -> [P, nk/2, 2, M]
        m = w.shape[1]
        return w.reshape(-1, 2, P, m).transpose(2, 0, 1, 3)

    W1e = inp["ln2_scale"][:, None] * inp["W1"]
    b1e = inp["b1"] + inp["ln2_bias"] @ inp["W1"]
    # tanh-form gelu: 0.5*g*(1+tanh(...)) - the 0.5 folds into w1a/b1a.
    # a-path: w1a = W1a*0.5*(SH2/SH0) so the final stt yields h2*SH2;
    # g-path: w1g = W1g*SG, descale 1/(SH0*SG) at the g tensor_scalar;
    # W2*S2, final descale 1/(SH2*S2). b1g stays raw (added post-descale).
    SA = (SH2 / SH0) if MLP_F8 else S1
    SGG = SG if MLP_F8 else S1
    W1a = W1e[:, :HID // 2] * (0.5 * SA)
    W1g = W1e[:, HID // 2:] * SGG
    b1a = (b1e[:HID // 2] * (0.5 * (SH2 if MLP_F8 else S1))).reshape(32, P).T
    b1g = b1e[HID // 2:].reshape(32, P).T
    b2row = (inp["b2"] * ((SH2 if MLP_F8 else S1) * S2)).reshape(1, D)
    w1a_t = np.stack([dr4(W1a[:, j * P:(j + 1) * P]) for j in range(32)])
    w1g_t = np.stack([dr4(W1g[:, j * P:(j + 1) * P]) for j in range(32)])
    W2s = inp["W2"] * S2
    w2_t = np.stack([dr4(W2s[:, mo * P:(mo + 1) * P]) for mo in range(DK)])
    bo_pc = inp["bo"].reshape(DK, P).T

    qm = q8 if MLP_F8 else bf
    qm1a = q8 if (MLP_F8 and W1A_F8) else bf
    qm2 = q8 if W2_F8 else bf
    w1a_8, w1g_8, w2_8 = qm1a(w1a_t), qm(w1g_t), qm2(w2_t)
    b1a_f, b1g_f, bo_f = f(b1a), f(b1g), f(bo_pc)
    b2_bf = bf(b2row)

    in_maps = []
    for c in range(N_CORES):
        b, l = c // 4, c % 4
        es = slice(EL * l, EL * (l + 1))
        idx = lane_idx(l)
        Wq_l, Wk_l, Wv_l = Wq2[:, es], Wk2[:, es], Wv2[:, es]
        in_maps.append({
            "xqT": xqT_all[b],
            "xkvT": xkvT_all[b],
            "augr_q": augr_q_all[b],
            "augr_kv": augr_kv_all[b],
            "wq": q8(dr4(Wq_l)), "wk": q8(dr4(Wk_l)), "wv": q8(dr4(Wv_l)),
            "aug_q": bf(np.stack([-Wq_l.sum(0), bq2[es]])),
            "aug_k": bf(np.stack([-Wk_l.sum(0), bk2[es]])),
            "aug_v": bf(np.stack([-Wv_l.sum(0), bv2[es]])),
            "wo": q8(Wo[es, :].reshape(2, P, D).transpose(1, 0, 2) * SWO),
            "bo_pc": bo_f,
            "xres_T": bf(inp["inputs_q"][b].T[:, idx]),
            "w1a_t": w1a_8, "w1g_t": w1g_8,
            "b1a_pc": b1a_f, "b1g_pc": b1g_f,
            "w2_t": w2_8,
            "b2row": b2_bf,
        })
    return in_maps


def unshard_output(results):
    """results: list of 8 dicts with 'out' [D, TL] -> full (B, NQ, D) f32."""
    full = np.empty((B, NQ, D), dtype=np.float32)
    for c in range(N_CORES):
        b, l = c // 4, c % 4
        full[b, lane_idx(l), :] = results[c]["out"].T
    return full


_NC_CACHE = {}


def _get_nc(n_iters=1):
    if n_iters not in _NC_CACHE:
        _NC_CACHE[n_iters] = build_kernel(n_iters)
    return _NC_CACHE[n_iters]


def kernel(**inputs) -> np.ndarray:
    nc = _get_nc(1)
    in_maps = prepare_inputs(inputs)
    res = run_bass_kernel_spmd(nc, in_maps, core_ids=list(range(N_CORES)))
    return unshard_output(res.results)
